# revision 1
# baseline (speedup 1.0000x reference)
"""RBF kernel ridge regression inference on 8 Trainium2 NeuronCores.

out[q] = sum_t exp(-gamma * ||X[q] - T[t]||^2) * coef[t]

Factored as exp(-g*x2[q]) * sum_t exp(2g*dot[t,q] - g*y2[t]) * coef[t] so the
whole inner computation maps onto TensorE (bf16 GEMM + matvec) and ScalarE
(one fused exp with per-partition bias).  Row norms are computed on DVE
(square+accumulate in one scalar_tensor_tensor op) so ScalarE runs Exp only
and never reloads its activation table.  Queries are sharded across the 8
cores; train_X and dual_coef are replicated.
"""

import numpy as np
import ml_dtypes

GAMMA = 1.0
N_QUERY, N_TRAIN, D = 8192, 8192, 512
N_CORES = 8
P = 128
QPC = N_QUERY // N_CORES  # 1024 queries per core
ND = D // P               # 4 contraction chunks
NT = N_TRAIN // P         # 64 train chunks
TGROUP = 8                # train chunks per resident tt DMA group
NTG = NT // TGROUP
QTILE = 512               # free dim of one sq-dist psum tile
NQC = QPC // QTILE        # 2 query chunks per core
NXC = QPC // P            # 8 query columns for x2 accumulation

_CACHE = {}


def _build_program(repeats=1):
    from contextlib import ExitStack

    import concourse.bass as bass
    import concourse.mybir as mybir
    import concourse.tile as tile
    from concourse import bacc

    f32 = mybir.dt.float32
    bf16 = mybir.dt.bfloat16
    AF = mybir.ActivationFunctionType
    MUL = mybir.AluOpType.mult

    nc = bacc.Bacc(
        "TRN2", target_bir_lowering=False, debug=False, num_devices=N_CORES
    )

    tt_d = nc.dram_tensor("tt_bf16", [D, N_TRAIN], bf16, kind="ExternalInput").ap()
    xt_d = nc.dram_tensor("xt_bf16", [D, QPC], bf16, kind="ExternalInput").ap()
    trf_d = nc.dram_tensor("train_f32", [N_TRAIN, D], f32, kind="ExternalInput").ap()
    xf_d = nc.dram_tensor("x_f32", [QPC, D], f32, kind="ExternalInput").ap()
    coef_d = nc.dram_tensor("coef_bf16", [P, NT], bf16, kind="ExternalInput").ap()
    out_d = nc.dram_tensor("out", [QPC], f32, kind="ExternalOutput").ap()
    x2r_d = nc.dram_tensor("x2_bounce", [QPC], f32).ap()  # internal scratch

    with tile.TileContext(nc) as tc, ExitStack() as ctx:
        res = ctx.enter_context(tc.tile_pool(name="res", bufs=1))
        ttp = ctx.enter_context(tc.tile_pool(name="ttp", bufs=1))
        stream = ctx.enter_context(tc.tile_pool(name="stream", bufs=4))
        exppool = ctx.enter_context(tc.tile_pool(name="expp", bufs=4))
        sqpool = ctx.enter_context(tc.tile_pool(name="psq", bufs=4, space="PSUM"))
        spool = ctx.enter_context(tc.tile_pool(name="pS", bufs=1, space="PSUM"))

        # ---- prologue: resident loads + x2 = rowwise ||X||^2 ----
        xt_sb = []
        for dc in range(ND):
            t = res.tile([P, QPC], bf16, tag=f"xt{dc}")
            nc.sync.dma_start(t[:], xt_d[dc * P : (dc + 1) * P, :])
            xt_sb.append(t)
        coef_sb = res.tile([P, NT], bf16, tag="coef")
        nc.sync.dma_start(coef_sb[:], coef_d[:])

        # x2 in column layout via DVE square+accumulate, then bounce through
        # DRAM to transpose into a single [1, QPC] row (hidden under main loop)
        x2_sb = res.tile([P, NXC], f32, tag="x2")
        for c in range(NXC):
            xtile = stream.tile([P, D], f32, tag="xf")
            nc.sync.dma_start(xtile[:], xf_d[c * P : (c + 1) * P, :])
            scr = stream.tile([P, D], bf16, tag="xscr")
            nc.vector.scalar_tensor_tensor(
                scr[:], xtile[:], 1.0, xtile[:], MUL, MUL,
                accum_out=x2_sb[:, c : c + 1],
            )
        nc.sync.dma_start(x2r_d.rearrange("(c p) -> p c", p=P), x2_sb[:])
        x2row = res.tile([1, QPC], f32, tag="x2row")
        nc.sync.dma_start(x2row[:], x2r_d.rearrange("(a q) -> a q", a=1))

        # ---- main loop over train chunks ----
        # S[qc] accumulates sum_t exp(...) * coef[t] as a [1, 512] psum row
        # per query chunk; each lives in its own psum bank so the long
        # accumulation groups never share a zero region.
        S_ps = [
            spool.tile([1, QTILE], f32, tag=f"S{qc}", name=f"S{qc}")
            for qc in range(NQC)
        ]
        for g in range(NTG):
            ttg = []
            for dc in range(ND):
                t = ttp.tile([P, TGROUP * P], bf16, tag=f"tt_{dc}_{g}")
                nc.sync.dma_start(
                    t[:],
                    tt_d[dc * P : (dc + 1) * P, g * TGROUP * P : (g + 1) * TGROUP * P],
                )
                ttg.append(t)
            for tl in range(TGROUP):
                ti = g * TGROUP + tl
                # y2n = -gamma * ||T[t]||^2 for this chunk (DVE, one op)
                trt = stream.tile([P, D], f32, tag="trf")
                nc.sync.dma_start(trt[:], trf_d[ti * P : (ti + 1) * P, :])
                scr2 = stream.tile([P, D], bf16, tag="trscr")
                y2nt = res.tile([P, 1], f32, tag=f"y2n_{ti}")
                nc.vector.scalar_tensor_tensor(
                    scr2[:], trt[:], -GAMMA, trt[:], MUL, MUL, accum_out=y2nt[:]
                )

                for qc in range(NQC):
                    ps = sqpool.tile([P, QTILE], f32, tag="sq")
                    for dc in range(ND):
                        nc.tensor.matmul(
                            ps[:],
                            ttg[dc][:, tl * P : (tl + 1) * P],
                            xt_sb[dc][:, qc * QTILE : (qc + 1) * QTILE],
                            start=(dc == 0),
                            stop=(dc == ND - 1),
                        )
                    et = exppool.tile([P, QTILE], bf16, tag="exp")
                    nc.scalar.activation(
                        et[:], ps[:], AF.Exp, bias=y2nt[:], scale=2.0 * GAMMA
                    )
                    nc.tensor.matmul(
                        S_ps[qc][:],
                        coef_sb[:, ti : ti + 1],
                        et[:],
                        start=(ti == 0),
                        stop=(ti == NT - 1),
                    )

        # ---- epilogue: out = exp(-g*x2) * S, all in row layout ----
        ex2 = res.tile([1, QPC], f32, tag="ex2")
        nc.scalar.activation(ex2[:], x2row[:], AF.Exp, scale=-GAMMA)
        outrow = res.tile([1, QPC], f32, tag="outrow")
        for qc in range(NQC):
            nc.vector.tensor_mul(
                outrow[:, qc * QTILE : (qc + 1) * QTILE],
                ex2[:, qc * QTILE : (qc + 1) * QTILE],
                S_ps[qc][:],
            )
        nc.sync.dma_start(out_d.rearrange("(a q) -> a q", a=1), outrow[:])

    nc.compile()
    return nc


def _get_program():
    if "nc" not in _CACHE:
        _CACHE["nc"] = _build_program()
    return _CACHE["nc"]


def make_in_maps(X, train_X, dual_coef):
    bf = ml_dtypes.bfloat16
    ttb = np.ascontiguousarray(train_X.T).astype(bf)
    coefb = np.ascontiguousarray(dual_coef.reshape(NT, P).T).astype(bf)
    XT = np.ascontiguousarray(X.T)
    in_maps = []
    for c in range(N_CORES):
        in_maps.append(
            {
                "tt_bf16": ttb,
                "xt_bf16": np.ascontiguousarray(XT[:, c * QPC : (c + 1) * QPC]).astype(
                    bf
                ),
                "train_f32": train_X,
                "x_f32": np.ascontiguousarray(X[c * QPC : (c + 1) * QPC]),
                "coef_bf16": coefb,
            }
        )
    return in_maps


def _get_callable():
    """Cached (fn, in_names, out_names, out_avals, zero_outs, mesh) for the
    sharded 8-core NEFF execution."""
    if "call" in _CACHE:
        return _CACHE["call"]

    import jax
    from jax.sharding import Mesh, PartitionSpec
    from jax.experimental.shard_map import shard_map

    import concourse.mybir as mybir
    from concourse import bass2jax
    from concourse.bass2jax import install_neuronx_cc_hook

    install_neuronx_cc_hook()
    nc = _get_program()

    partition_name = (
        nc.partition_id_tensor.name if nc.partition_id_tensor else None
    )
    in_names, out_names, out_avals, zero_outs = [], [], [], []
    for alloc in nc.m.functions[0].allocations:
        if not isinstance(alloc, mybir.MemoryLocationSet):
            continue
        if alloc.kind not in ("ExternalInput", "ExternalOutput"):
            continue
        name = alloc.memorylocations[0].name
        if alloc.kind == "ExternalInput":
            if name != partition_name:
                in_names.append(name)
        else:
            out_names.append(name)
            shape = tuple(alloc.tensor_shape)
            dtype = mybir.dt.np(alloc.dtype)
            out_avals.append(jax.core.ShapedArray(shape, dtype))
            zero_outs.append(np.zeros(shape, dtype))
    all_in_names = in_names + out_names
    if partition_name is not None:
        all_in_names = all_in_names + [partition_name]

    def _body(*args):
        operands = list(args)
        if partition_name is not None:
            operands.append(bass2jax.partition_id_tensor())
        outs = bass2jax._bass_exec_p.bind(
            *operands,
            out_avals=tuple(out_avals),
            in_names=tuple(all_in_names),
            out_names=tuple(out_names),
            lowering_input_output_aliases=(),
            sim_require_finite=True,
            sim_require_nnan=True,
            nc=nc,
        )
        return tuple(outs)

    devices = jax.devices()[:N_CORES]
    mesh = Mesh(np.asarray(devices), ("core",))
    n_all = len(in_names) + len(out_names)
    fn = jax.jit(
        shard_map(
            _body,
            mesh=mesh,
            in_specs=(PartitionSpec("core"),) * n_all,
            out_specs=(PartitionSpec("core"),) * len(out_names),
            check_rep=False,
        ),
        keep_unused=True,
    )
    _CACHE["call"] = (fn, in_names, out_names, out_avals, zero_outs, mesh)
    return _CACHE["call"]


def concat_inputs(in_maps):
    fn, in_names, out_names, out_avals, zero_outs, mesh = _get_callable()
    concat_in = [
        np.concatenate([np.asarray(m[name]) for m in in_maps], axis=0)
        for name in in_names
    ]
    concat_zeros = [
        np.zeros((N_CORES * z.shape[0], *z.shape[1:]), z.dtype) for z in zero_outs
    ]
    return concat_in + concat_zeros


def kernel(X, train_X, dual_coef):
    X = np.asarray(X, dtype=np.float32)
    train_X = np.asarray(train_X, dtype=np.float32)
    dual_coef = np.asarray(dual_coef, dtype=np.float32)

    fn, in_names, out_names, out_avals, zero_outs, mesh = _get_callable()
    in_maps = make_in_maps(X, train_X, dual_coef)
    args = concat_inputs(in_maps)
    outs = fn(*args)
    out = np.asarray(outs[0]).reshape(-1)
    return out.astype(np.float32)



# revision 3
# speedup vs baseline: 1.0626x; 1.0626x over previous
"""RBF kernel ridge regression inference on 8 Trainium2 NeuronCores.

out[q] = sum_t exp(-gamma * ||X[q] - T[t]||^2) * coef[t]

Factored with the query index on PSUM partitions and the train index on the
free axis:

    out[q] = sum_t exp(2g*dot[q,t] - g*x2[q]) * w2[t],   w2[t] = coef[t]*exp(-g*y2[t])

so the whole inner loop is: fp8 DoubleRow GEMM (256-deep contraction per
matmul, 2x PE throughput) -> one fused ScalarE exp per [128,2048] psum tile
(bias = -g*x2 per partition) -> one DVE tensor_tensor_reduce that multiplies
by w2[t] and reduces along t.  No coef-matvec on the tensor engine at all.

Sharding: a 4 query-group x 2 train-group grid over the 8 cores; the two
train partials per query group are summed on the host (32KB of adds).
Row norms x2/y2 run on DVE from bf16 row layouts; the w2 row is built once
per train half and replicated across partitions with a broadcast DMA.
"""

import numpy as np
import ml_dtypes

GAMMA = 1.0
N_QUERY, N_TRAIN, D = 8192, 8192, 512
N_CORES = 8
P = 128
QG, TG = 4, 2             # query groups x train groups = 8 cores
QPC = N_QUERY // QG       # 2048 queries per core
TPC = N_TRAIN // TG       # 4096 train points per core
NQC = QPC // P            # 16 query chunks of 128
TT = 2048                 # free width of one psum tile
NTCG = TPC // TT          # 2 train tiles per core
NDR = 2                   # DoubleRow contraction groups (2 x 256 = 512)

_CACHE = {}


def _build_program():
    from contextlib import ExitStack

    import concourse.bass as bass
    import concourse.mybir as mybir
    import concourse.tile as tile
    from concourse import bacc

    f32 = mybir.dt.float32
    bf16 = mybir.dt.bfloat16
    f8 = mybir.dt.float8e4
    AF = mybir.ActivationFunctionType
    MUL = mybir.AluOpType.mult
    ADD = mybir.AluOpType.add
    DR = mybir.MatmulPerfMode.DoubleRow

    nc = bacc.Bacc(
        "TRN2", target_bir_lowering=False, debug=False, num_devices=N_CORES
    )

    # DRAM inputs (all host-pre-laid-out; see make_in_maps)
    tt_d = [
        nc.dram_tensor(f"tt{g}", [P, NDR, TPC], f8, kind="ExternalInput").ap()
        for g in range(NDR)
    ]
    x_d = [
        nc.dram_tensor(f"x{g}", [P, NDR, QPC], f8, kind="ExternalInput").ap()
        for g in range(NDR)
    ]
    tr_d = [
        nc.dram_tensor(f"tr{b}", [P, 8, D], bf16, kind="ExternalInput").ap()
        for b in range(TPC // (8 * P))
    ]
    xr_d = [
        nc.dram_tensor(f"xr{b}", [P, 8, D], bf16, kind="ExternalInput").ap()
        for b in range(QPC // (8 * P))
    ]
    coef_d = nc.dram_tensor("coefc", [P, TPC // P], f32, kind="ExternalInput").ap()
    out_d = nc.dram_tensor("out", [QPC], f32, kind="ExternalOutput").ap()
    w2_d = nc.dram_tensor("w2_bounce", [TPC], bf16).ap()  # internal scratch

    with tile.TileContext(nc) as tc, ExitStack() as ctx:
        res = ctx.enter_context(tc.tile_pool(name="res", bufs=1))
        stream = ctx.enter_context(tc.tile_pool(name="stream", bufs=3))
        etp = ctx.enter_context(tc.tile_pool(name="etp", bufs=4))
        scrp = ctx.enter_context(tc.tile_pool(name="scrp", bufs=2))
        psq = ctx.enter_context(tc.tile_pool(name="psq", bufs=2, space="PSUM"))

        # ---- resident GEMM operands ----
        x_sb, tt_sb = [], []
        for g in range(NDR):
            t = res.tile([P, NDR, QPC], f8, tag=f"x{g}")
            nc.sync.dma_start(t[:], x_d[g])
            x_sb.append(t)
        for g in range(NDR):
            t = res.tile([P, NDR, TPC], f8, tag=f"tt{g}")
            # split per train half so the first matmuls start sooner
            for tcg in range(NTCG):
                nc.sync.dma_start(
                    t[:, :, tcg * TT : (tcg + 1) * TT],
                    tt_d[g][:, :, tcg * TT : (tcg + 1) * TT],
                )
            tt_sb.append(t)
        coefc = res.tile([P, TPC // P], f32, tag="coefc")
        nc.sync.dma_start(coefc[:], coef_d)

        # ---- x2 = -gamma * ||X[q]||^2 in [128, NQC] column layout ----
        nx2 = res.tile([P, NQC], f32, tag="nx2")
        for b in range(len(xr_d)):
            xrt = stream.tile([P, 8, D], bf16, tag="xr")
            nc.sync.dma_start(xrt[:], xr_d[b])
            xscr = stream.tile([P, 8, D], bf16, tag="xscr")
            for j in range(8):
                nc.vector.scalar_tensor_tensor(
                    xscr[:, j : j + 1, :],
                    xrt[:, j : j + 1, :],
                    -GAMMA,
                    xrt[:, j : j + 1, :],
                    MUL,
                    MUL,
                    accum_out=nx2[:, 8 * b + j : 8 * b + j + 1],
                )

        # ---- y2 -> w2 = coef * exp(-gamma*y2), replicated to all partitions ----
        ny2 = res.tile([P, TPC // P], f32, tag="ny2")
        ecol = res.tile([P, TPC // P], f32, tag="ecol")
        w2col = res.tile([P, TPC // P], bf16, tag="w2col")
        w2rep = res.tile([P, TPC], bf16, tag="w2rep")
        NB = 8 * P  # train rows per tr block
        for tcg in range(NTCG):
            for b in range(tcg * TT // NB, (tcg + 1) * TT // NB):
                trt = stream.tile([P, 8, D], bf16, tag="tr")
                nc.sync.dma_start(trt[:], tr_d[b])
                tscr = stream.tile([P, 8, D], bf16, tag="tscr")
                for j in range(8):
                    nc.vector.scalar_tensor_tensor(
                        tscr[:, j : j + 1, :],
                        trt[:, j : j + 1, :],
                        -GAMMA,
                        trt[:, j : j + 1, :],
                        MUL,
                        MUL,
                        accum_out=ny2[:, 8 * b + j : 8 * b + j + 1],
                    )
            cs = slice(tcg * TT // P, (tcg + 1) * TT // P)
            nc.scalar.activation(ecol[:, cs], ny2[:, cs], AF.Exp)
            nc.vector.tensor_mul(w2col[:, cs], ecol[:, cs], coefc[:, cs])
            nc.sync.dma_start(
                w2_d[tcg * TT : (tcg + 1) * TT].rearrange("(c p) -> p c", p=P),
                w2col[:, cs],
            )
            nc.sync.dma_start(
                w2rep[:, tcg * TT : (tcg + 1) * TT],
                w2_d[tcg * TT : (tcg + 1) * TT]
                .rearrange("(a t) -> a t", a=1)
                .partition_broadcast(P),
            )

        # ---- main loop ----
        Sa = res.tile([P, NQC], f32, tag="Sa")
        Sb = res.tile([P, NQC], f32, tag="Sb")
        S_parts = [Sa, Sb]
        for qc in range(NQC):
            for tcg in range(NTCG):
                ps = psq.tile([P, TT], f32, tag="ps")
                for g in range(NDR):
                    for j in range(TT // 512):
                        nc.tensor.matmul(
                            ps[:, j * 512 : (j + 1) * 512],
                            x_sb[g][:, :, qc * P : (qc + 1) * P],
                            tt_sb[g][:, :, tcg * TT + j * 512 : tcg * TT + (j + 1) * 512],
                            start=(g == 0),
                            stop=(g == NDR - 1),
                            perf_mode=DR,
                        )
                et = etp.tile([P, TT], bf16, tag="et")
                nc.scalar.activation(
                    et[:], ps[:], AF.Exp,
                    bias=nx2[:, qc : qc + 1], scale=2.0 * GAMMA,
                )
                scr = scrp.tile([P, TT], bf16, tag="scr")
                nc.vector.scalar_tensor_tensor(
                    scr[:],
                    et[:],
                    1.0,
                    w2rep[:, tcg * TT : (tcg + 1) * TT],
                    MUL,
                    MUL,
                    accum_out=S_parts[tcg][:, qc : qc + 1],
                )

        # ---- epilogue: out = Sa + Sb, row-majorized via DMA ----
        S16 = res.tile([P, NQC], f32, tag="S16")
        nc.vector.tensor_add(S16[:], Sa[:], Sb[:])
        nc.sync.dma_start(out_d.rearrange("(c p) -> p c", p=P), S16[:])

    nc.compile()
    return nc


def _get_program():
    if "nc" not in _CACHE:
        _CACHE["nc"] = _build_program()
    return _CACHE["nc"]


def make_in_maps(X, train_X, dual_coef):
    bf = ml_dtypes.bfloat16
    f8 = ml_dtypes.float8_e4m3

    def dr_layout(A):
        # A: [n, D] rows -> [g][P, 2, n] with d = 256*g + 128*i + p
        At = np.ascontiguousarray(A.T).reshape(NDR, 2, P, A.shape[0])
        return [np.ascontiguousarray(At[g].transpose(1, 0, 2)).astype(f8)
                for g in range(NDR)]

    def row_blocks(A):
        # A: [n, D] -> [n//(8P)][P, 8, D] with row = b*8P + j*P + p
        n = A.shape[0]
        R = A.reshape(n // (8 * P), 8, P, D)
        return [np.ascontiguousarray(R[b].transpose(1, 0, 2)).astype(bf)
                for b in range(n // (8 * P))]

    in_maps = []
    for c in range(N_CORES):
        i, j = c // TG, c % TG
        Xs = X[i * QPC : (i + 1) * QPC]
        Ts = train_X[j * TPC : (j + 1) * TPC]
        cs = dual_coef[j * TPC : (j + 1) * TPC]
        m = {}
        for g, arr in enumerate(dr_layout(Xs)):
            m[f"x{g}"] = arr
        for g, arr in enumerate(dr_layout(Ts)):
            m[f"tt{g}"] = arr
        for b, arr in enumerate(row_blocks(Ts)):
            m[f"tr{b}"] = arr
        for b, arr in enumerate(row_blocks(Xs)):
            m[f"xr{b}"] = arr
        m["coefc"] = np.ascontiguousarray(
            cs.reshape(TPC // P, P).T
        ).astype(np.float32)
        in_maps.append(m)
    return in_maps


def _get_callable():
    """Cached (fn, in_names, out_names, out_avals, zero_outs, mesh) for the
    sharded 8-core NEFF execution."""
    if "call" in _CACHE:
        return _CACHE["call"]

    import jax
    from jax.sharding import Mesh, PartitionSpec
    from jax.experimental.shard_map import shard_map

    import concourse.mybir as mybir
    from concourse import bass2jax
    from concourse.bass2jax import install_neuronx_cc_hook

    install_neuronx_cc_hook()
    nc = _get_program()

    partition_name = (
        nc.partition_id_tensor.name if nc.partition_id_tensor else None
    )
    in_names, out_names, out_avals, zero_outs = [], [], [], []
    for alloc in nc.m.functions[0].allocations:
        if not isinstance(alloc, mybir.MemoryLocationSet):
            continue
        if alloc.kind not in ("ExternalInput", "ExternalOutput"):
            continue
        name = alloc.memorylocations[0].name
        if alloc.kind == "ExternalInput":
            if name != partition_name:
                in_names.append(name)
        else:
            out_names.append(name)
            shape = tuple(alloc.tensor_shape)
            dtype = mybir.dt.np(alloc.dtype)
            out_avals.append(jax.core.ShapedArray(shape, dtype))
            zero_outs.append(np.zeros(shape, dtype))
    all_in_names = in_names + out_names
    if partition_name is not None:
        all_in_names = all_in_names + [partition_name]

    def _body(*args):
        operands = list(args)
        if partition_name is not None:
            operands.append(bass2jax.partition_id_tensor())
        outs = bass2jax._bass_exec_p.bind(
            *operands,
            out_avals=tuple(out_avals),
            in_names=tuple(all_in_names),
            out_names=tuple(out_names),
            lowering_input_output_aliases=(),
            sim_require_finite=True,
            sim_require_nnan=True,
            nc=nc,
        )
        return tuple(outs)

    devices = jax.devices()[:N_CORES]
    mesh = Mesh(np.asarray(devices), ("core",))
    n_all = len(in_names) + len(out_names)
    fn = jax.jit(
        shard_map(
            _body,
            mesh=mesh,
            in_specs=(PartitionSpec("core"),) * n_all,
            out_specs=(PartitionSpec("core"),) * len(out_names),
            check_rep=False,
        ),
        keep_unused=True,
    )
    _CACHE["call"] = (fn, in_names, out_names, out_avals, zero_outs, mesh)
    return _CACHE["call"]


def concat_inputs(in_maps):
    fn, in_names, out_names, out_avals, zero_outs, mesh = _get_callable()
    concat_in = [
        np.concatenate([np.asarray(m[name]) for m in in_maps], axis=0)
        for name in in_names
    ]
    concat_zeros = [
        np.zeros((N_CORES * z.shape[0], *z.shape[1:]), z.dtype) for z in zero_outs
    ]
    return concat_in + concat_zeros


def kernel(X, train_X, dual_coef):
    X = np.asarray(X, dtype=np.float32)
    train_X = np.asarray(train_X, dtype=np.float32)
    dual_coef = np.asarray(dual_coef, dtype=np.float32)

    fn, in_names, out_names, out_avals, zero_outs, mesh = _get_callable()
    in_maps = make_in_maps(X, train_X, dual_coef)
    args = concat_inputs(in_maps)
    outs = fn(*args)
    # per-core partials: core c = (qgroup i = c//TG, train half j = c%TG)
    parts = np.asarray(outs[0]).reshape(QG, TG, QPC)
    out = parts.sum(axis=1).reshape(-1)
    return out.astype(np.float32)
